# revision 27
# baseline (speedup 1.0000x reference)
"""AttentionAggregator Trainium2 kernel (8-core SPMD, data-parallel over nodes).

Math (per node b with neighbors n):
  x_att   = lrelu_.01(x @ W_att);  neib_att = lrelu_.01(neibs @ W_att)
  e[b,n]  = lrelu_.2(x_att[b]@a_x + neib_att[b,n]@a_n)
  att     = softmax_n(e)
  agg[b]  = sum_n att[b,n] * neibs[b,n]
  out     = relu([x@W_fcx, agg@W_fcn])

Score rewrite (host-side, exact in the weights):
  sum_h a_h*lrelu(z_h) = sum_{seg1} relu(x.col) - sum_{seg2} relu(x.col)
  over 258 columns (relu-pair form), via lrelu(u)=.01u+.99relu(u),
  a*lrelu(z)=sign(a)*lrelu(|a|z), k*relu(u)=relu(k*u), u=relu(u)-relu(-u).

v2 design:
  - neibs cast to bf16 on host and laid out p-major per 128-node block so
    the natural load is fully contiguous per partition.
  - transposed neibs tiles come from the DMA xbar transpose (bf16) straight
    from HBM -> no PE transposes, no PSUM->SBUF tile copies.
  - x shipped pre-transposed bf16 from host (used for logits and fc).
  - all matmuls bf16 (FWL weight loads, 1 cycle/row): scores, agg, fc.
  - score relu+/-accumulate drains rotated across DVE/Pool/ACT (greedy
    load balance); softmax in a transposed [T,128] layout, bf16 where it
    doesn't hurt; outputs fp32.
"""
import warnings
warnings.filterwarnings("ignore")
import numpy as np
import ml_dtypes
from contextlib import ExitStack

import concourse.bass as bass
import concourse.tile as tile
from concourse import bacc, mybir, masks
from concourse.bass_utils import run_bass_kernel_spmd

F32 = mybir.dt.float32
BF16 = mybir.dt.bfloat16
AF = mybir.ActivationFunctionType
ALU = mybir.AluOpType
AX = mybir.AxisListType

N_CORES = 8
B_FULL, NB, D, H, O = 20000, 32, 128, 256, 128
HW6 = H + 2  # 258 score columns


def _score_weights(W_att: np.ndarray, a_half: np.ndarray):
    """Build the 258-column relu-pair score weight matrix. Returns (W6, split)."""
    pos = np.where(a_half >= 0)[0]
    neg = np.where(a_half < 0)[0]
    Wabs = W_att * np.abs(a_half)[None, :]
    w_d = (W_att @ a_half).astype(np.float64)
    seg1 = np.concatenate([0.99 * Wabs[:, pos], 0.01 * w_d[:, None]], axis=1)
    seg2 = np.concatenate([0.99 * Wabs[:, neg], -0.01 * w_d[:, None]], axis=1)
    W6 = np.concatenate([seg1, seg2], axis=1).astype(np.float32)
    return W6, seg1.shape[1]


def _blocks(bc):
    out = []
    o = 0
    while o < bc:
        f = min(128, bc - o)
        assert f * NB % 128 == 0
        out.append((o, f))
        o += f
    return out


_PROG_CACHE = {}

# test-harness knobs (harness calls kernel() with defaults: no tracing)
TRACE = False
TRACE_DIR = None
LAST_RESULTS = None


def _drain_engines(T):
    """Greedy per-tile drain-engine assignment balancing per-block load.

    "V": single fused drain on DVE (PSUM-capable).
    "A": two-segment Relu drain on ACT (accum+ / accum-).
    Pool has no PSUM port and no free-axis accumulate, so it only gets
    SBUF-side softmax work. Costs (ns) per drain; handicaps model each
    engine's other per-block duties.
    """
    load = {"V": 2000.0, "A": 1300.0}
    cost = {"V": 480.0, "A": 1175.0}
    plan = []
    for _ in range(T):
        e = min(load, key=lambda k: load[k] + cost[k])
        load[e] += cost[e]
        plan.append(e)
    return plan


def _build_program(bc, split_n, split_x, n_cores=N_CORES):
    """Build + compile the SPMD program for bc nodes per core."""
    key = (bc, split_n, split_x, n_cores)
    if key in _PROG_CACHE:
        return _PROG_CACHE[key]

    nc = bacc.Bacc("TRN2", target_bir_lowering=False, debug=False,
                   num_devices=n_cores)

    ne_d = nc.dram_tensor("ne", [bc * NB, D], BF16, kind="ExternalInput").ap()
    netm_d = nc.dram_tensor("netm", [bc * NB, D], BF16, kind="ExternalInput").ap()
    xt_d = nc.dram_tensor("xt", [D, bc], BF16, kind="ExternalInput").ap()
    w6n_d = nc.dram_tensor("w6n", [D, HW6], BF16, kind="ExternalInput").ap()
    w6x_d = nc.dram_tensor("w6x", [D, HW6], BF16, kind="ExternalInput").ap()
    wfcx_d = nc.dram_tensor("wfcx", [D, O], BF16, kind="ExternalInput").ap()
    wfcn_d = nc.dram_tensor("wfcn", [D, O], BF16, kind="ExternalInput").ap()
    mask_d = nc.dram_tensor("mask", [128, 4], BF16, kind="ExternalInput").ap()
    mask4_d = nc.dram_tensor("mask4", [128, 4], BF16, kind="ExternalInput").ap()
    psel_d = nc.dram_tensor("psel", [128, 32], BF16, kind="ExternalInput").ap()
    cful_d = nc.dram_tensor("cful", [128, HW6], F32, kind="ExternalInput").ap()
    cfux_d = nc.dram_tensor("cfux", [128, HW6], F32, kind="ExternalInput").ap()
    out_d = nc.dram_tensor("out", [bc, 2 * O], F32, kind="ExternalOutput").ap()

    with tile.TileContext(nc) as tc, ExitStack() as ctx:
        consts = ctx.enter_context(tc.tile_pool(name="consts", bufs=1))
        nepool = ctx.enter_context(tc.tile_pool(name="ne", bufs=3))
        ntpool = ctx.enter_context(tc.tile_pool(name="nt", bufs=3))
        xtpool = ctx.enter_context(tc.tile_pool(name="xtp", bufs=2))
        sc_v = ctx.enter_context(tc.tile_pool(name="scr_v", bufs=2))
        sc_p = ctx.enter_context(tc.tile_pool(name="scr_p", bufs=2))
        sc_a = ctx.enter_context(tc.tile_pool(name="scr_a", bufs=2))
        blkpool = ctx.enter_context(tc.tile_pool(name="blk", bufs=2))
        ps_sc = ctx.enter_context(tc.tile_pool(name="ps_sc", bufs=5, space="PSUM"))
        ps_agg = ctx.enter_context(tc.tile_pool(name="ps_agg", bufs=1, space="PSUM"))
        ps_misc = ctx.enter_context(tc.tile_pool(name="ps_misc", bufs=2, space="PSUM"))

        identf = consts.tile([128, 128], F32)
        masks.make_identity(nc, identf[:])
        ident = consts.tile([128, 128], BF16)
        nc.vector.tensor_copy(ident[:], identf[:])
        w6n = consts.tile([D, HW6], BF16)
        w6x = consts.tile([D, HW6], BF16)
        wfcx = consts.tile([D, O], BF16)
        wfcn = consts.tile([D, O], BF16)
        mask = consts.tile([128, 4], BF16)
        mask4 = consts.tile([128, 4], BF16)
        psel = consts.tile([128, 32], BF16)
        cful = consts.tile([128, HW6], F32)
        cfux = consts.tile([128, HW6], F32)
        for t, dd in [(w6n, w6n_d), (w6x, w6x_d), (wfcx, wfcx_d),
                      (wfcn, wfcn_d), (mask, mask_d), (mask4, mask4_d),
                      (psel, psel_d), (cful, cful_d), (cfux, cfux_d)]:
            nc.sync.dma_start(t[:], dd)

        def block_setup(boff, F):
            """DMAs + x-side for a block; returns block state."""
            T = F * NB // 128  # score tiles in this block
            rbase = boff * NB

            # natural (p-major host layout): partition p <- rows p*T..p*T+T
            ne_buf = nepool.tile([128, 32 * D], BF16, tag="ne")
            ne_v = ne_buf[:].rearrange("p (t d) -> p t d", d=D)
            nc.sync.dma_start(
                ne_v[:, :T, :],
                ne_d[rbase: rbase + 128 * T, :].rearrange(
                    "(p t) d -> p t d", t=T))

            # transposed via DMA xbar (ACT HWDGE ring, parallel to sync ring)
            # from the tile-major copy: SBUF col (t*128+p) <- dram row 128t+p.
            nt_buf = ntpool.tile([128, 32 * 128], BF16, tag="nt")
            nt_v = nt_buf[:].rearrange("d (t p) -> d t p", p=128)
            nc.scalar.dma_start_transpose(
                nt_v[:, :T, :], netm_d[rbase: rbase + 128 * T, :])

            # ---- x side (xT shipped pre-transposed bf16)
            xtr = xtpool.tile([D, 128], BF16, tag="xtr")
            nc.sync.dma_start(xtr[:, :F], xt_d[:, boff:boff + F])
            xs_ps = ps_misc.tile([128, 258], F32, tag="misc")
            nc.tensor.matmul(xs_ps[:F, :], xtr[:, :F], w6x[:], start=True, stop=True)
            xscr = sc_v.tile([128, HW6], F32, tag="scr_v")
            sx = blkpool.tile([128, 1], F32, tag="sx")
            nc.vector.scalar_tensor_tensor(
                xscr[:F, :], xs_ps[:F, :], 0.0, cfux[:F, :],
                op0=ALU.max, op1=ALU.mult, accum_out=sx[:F, :])
            sx4 = blkpool.tile([128, 4], BF16, tag="sx4")
            nc.gpsimd.tensor_scalar(sx4[:F, :], mask4[:F, :], sx[:F, 0:1], None,
                                    op0=ALU.mult)

            spos = blkpool.tile([128, 32], F32, tag="spos")
            sneg = blkpool.tile([128, 32], F32, tag="sneg")
            nc.gpsimd.memset(sneg[:, :T], 0.0)
            agg_ps = ps_agg.tile([128, 128], F32, tag="agg")
            return dict(ne_v=ne_v, nt_v=nt_v, xtr=xtr, T=T, F=F, boff=boff,
                        spos=spos, sneg=sneg, sx4=sx4, agg_ps=agg_ps)

        def emit_score(bs, t):
            s_ps = ps_sc.tile([128, HW6], F32, tag="sc")
            nc.tensor.matmul(s_ps[:], bs["nt_v"][:, t, :], w6n[:],
                             start=True, stop=True)
            if bs["plan"][t] == "V":
                scr = sc_v.tile([128, HW6], F32, tag="scr_v")
                nc.vector.scalar_tensor_tensor(
                    scr[:], s_ps[:], 0.0, cful[:],
                    op0=ALU.max, op1=ALU.mult,
                    accum_out=bs["spos"][:, t:t + 1])
            else:
                scr = sc_a.tile([128, HW6], BF16, tag="scr_a")
                nc.scalar.activation(scr[:, :split_n], s_ps[:, :split_n],
                                     AF.Relu, accum_out=bs["spos"][:, t:t + 1])
                nc.scalar.activation(scr[:, split_n:HW6],
                                     s_ps[:, split_n:HW6], AF.Relu,
                                     accum_out=bs["sneg"][:, t:t + 1])

        def emit_agg(bs, t, a_all_h, t0):
            a_v = a_all_h[:].rearrange("p (t j) -> p t j", j=4)
            nc.tensor.matmul(bs["agg_ps"][:, 4 * t:4 * (t + 1)],
                             bs["ne_v"][:, t, :], a_v[:, t - t0, :],
                             start=True, stop=True)

        def softmax_half(bs, t0, TH):
            """Softmax for tiles [t0, t0+TH); returns a_all for the half."""
            F, spos, sneg = bs["F"], bs["spos"], bs["sneg"]
            sxg_ps = ps_misc.tile([128, 258], F32, tag="misc")
            nc.tensor.matmul(sxg_ps[:TH, 0:4], psel[:F, t0:t0 + TH],
                             bs["sx4"][:F, :], start=True, stop=True)
            sxg = blkpool.tile([16, 4], F32, tag="sxg")
            nc.vector.tensor_copy(sxg[:TH, :], sxg_ps[:TH, 0:4])
            s_col = blkpool.tile([128, 16], BF16, tag="s_col")
            nc.gpsimd.tensor_tensor(s_col[:, :TH], spos[:, t0:t0 + TH],
                                    sneg[:, t0:t0 + TH], op=ALU.subtract)
            snt_ps = ps_misc.tile([128, 258], BF16, tag="misc")
            nc.tensor.transpose(snt_ps[:TH, :128], s_col[:, :TH], ident[:])
            z = blkpool.tile([16, 128], F32, tag="z")
            nc.vector.tensor_tensor(
                z[:TH, :].rearrange("t (j n) -> t j n", n=32),
                snt_ps[:TH, :128].rearrange("t (j n) -> t j n", n=32),
                sxg[:TH, :].unsqueeze(2).broadcast_to([TH, 4, 32]),
                op=ALU.add)
            zl = blkpool.tile([16, 128], F32, tag="zl")
            nc.vector.scalar_tensor_tensor(zl[:TH, :], z[:TH, :], 0.2,
                                           z[:TH, :], op0=ALU.mult, op1=ALU.max)
            ex = blkpool.tile([16, 128], F32, tag="ex")
            nc.scalar.activation(ex[:TH, :], zl[:TH, :], AF.Exp)
            sums = blkpool.tile([16, 4], F32, tag="sums")
            nc.vector.tensor_reduce(
                sums[:TH, :], ex[:TH, :].rearrange("t (j n) -> t j n", n=32),
                axis=AX.X, op=ALU.add)
            rec = blkpool.tile([16, 4], F32, tag="rec")
            nc.vector.reciprocal(rec[:TH, :], sums[:TH, :])
            att = blkpool.tile([16, 128], BF16, tag="att")
            nc.gpsimd.tensor_tensor(
                att[:TH, :].rearrange("t (j n) -> t j n", n=32),
                ex[:TH, :].rearrange("t (j n) -> t j n", n=32),
                rec[:TH, :].unsqueeze(2).broadcast_to([TH, 4, 32]),
                op=ALU.mult)
            att_ps = ps_misc.tile([128, 258], BF16, tag="misc")
            nc.tensor.transpose(att_ps[:, :TH], att[:TH, :], ident[:TH, :TH])
            a_all = blkpool.tile([128, 64], BF16, tag="a_all")
            nc.vector.tensor_tensor(
                a_all[:].rearrange("p (t j) -> p t j", j=4)[:, :TH, :],
                mask[:].unsqueeze(1).broadcast_to([128, TH, 4]),
                att_ps[:, :TH].unsqueeze(2).broadcast_to([128, TH, 4]),
                op=ALU.mult)
            return a_all

        def block_finish(bs):
            F, boff = bs["F"], bs["boff"]
            aggt = blkpool.tile([D, 128], BF16, tag="aggt")
            nc.scalar.copy(aggt[:, :F], bs["agg_ps"][:, :F])
            fc_ps = ps_misc.tile([128, 258], F32, tag="misc")
            nc.tensor.matmul(fc_ps[:F, 0:O], bs["xtr"][:, :F], wfcx[:],
                             start=True, stop=True)
            nc.tensor.matmul(fc_ps[:F, O:2 * O], aggt[:, :F], wfcn[:],
                             start=True, stop=True)
            out_sb = blkpool.tile([128, 2 * O], F32, tag="out")
            nc.scalar.activation(out_sb[:F, :], fc_ps[:F, :2 * O], AF.Relu)
            nc.sync.dma_start(out_d[boff:boff + F, :], out_sb[:F, :])

        # ---- half-block software pipeline
        def halves(T):
            h0 = (T + 1) // 2
            return [(0, h0), (h0, T - h0)]

        blocks = _blocks(bc)
        steps = []  # (block_state, t0, TH)
        for (boff, F) in blocks:
            T = F * NB // 128
            bs = None  # created lazily at first half
            for (t0, TH) in halves(T):
                steps.append((boff, F, t0, TH))

        bs_by_off = {}
        pend_smax = None   # (bs, t0, TH) awaiting softmax emission
        pend_agg = None    # (bs, t0, TH, a_all) awaiting agg interleave
        for (boff, F, t0, TH) in steps:
            if boff not in bs_by_off:
                bs = block_setup(boff, F)
                bs["plan"] = _drain_engines(bs["T"])
                bs["done_halves"] = 0
                bs_by_off = {boff: bs}  # keep only current (states referenced in pend_*)
            bs = bs_by_off[boff]
            agg_job = pend_agg
            pend_agg = None
            for i in range(TH):
                emit_score(bs, t0 + i)
                if agg_job is not None:
                    abs_, at0, aTH, a_all = agg_job
                    if i < aTH:
                        emit_agg(abs_, at0 + i, a_all, at0)
                    if i == TH - 1:
                        for j in range(TH, aTH):
                            emit_agg(abs_, at0 + j, a_all, at0)
            if agg_job is not None:
                abs_, at0, aTH, _ = agg_job
                abs_["done_halves"] += 1
                if abs_["done_halves"] == 2:
                    block_finish(abs_)
            if pend_smax is not None:
                sbs, st0, sTH = pend_smax
                a_all = softmax_half(sbs, st0, sTH)
                pend_agg = (sbs, st0, sTH, a_all)
            pend_smax = (bs, t0, TH)
        # drain the pipeline tail
        if pend_smax is not None:
            sbs, st0, sTH = pend_smax
            a_all = softmax_half(sbs, st0, sTH)
            if pend_agg is not None:
                abs_, at0, aTH, pa = pend_agg
                for j in range(aTH):
                    emit_agg(abs_, at0 + j, pa, at0)
                abs_["done_halves"] += 1
                if abs_["done_halves"] == 2:
                    block_finish(abs_)
            for j in range(sTH):
                emit_agg(sbs, st0 + j, a_all, st0)
            sbs["done_halves"] += 1
            if sbs["done_halves"] == 2:
                block_finish(sbs)

    nc.compile()
    _PROG_CACHE[key] = nc
    return nc


def _permute_pmajor(ne_c: np.ndarray, bc: int) -> np.ndarray:
    """Per 128-node block, reorder rows tile-major -> partition-major."""
    chunks = []
    r = 0
    for (boff, F) in _blocks(bc):
        T = F * NB // 128
        blk = ne_c[r:r + 128 * T]  # rows ordered (t, p)
        chunks.append(blk.reshape(T, 128, D).transpose(1, 0, 2).reshape(-1, D))
        r += 128 * T
    return np.concatenate(chunks, axis=0)


def kernel(x, neibs, W_att, W_fcx, W_fcn, a, n_cores=N_CORES):
    x = np.asarray(x, dtype=np.float32)
    neibs = np.asarray(neibs, dtype=np.float32)
    W_att = np.asarray(W_att, dtype=np.float32)
    W_fcx = np.asarray(W_fcx, dtype=np.float32)
    W_fcn = np.asarray(W_fcn, dtype=np.float32)
    a = np.asarray(a, dtype=np.float32)

    B = x.shape[0]
    bc = B // n_cores
    a_x, a_n = a[:H, 0], a[H:, 0]
    w6x_np, split_x = _score_weights(W_att, a_x)
    w6n_np, split_n = _score_weights(W_att, a_n)
    mask_np = np.equal.outer(np.arange(128) // 32, np.arange(4))
    mask4_np = np.equal.outer(np.arange(128) % 4, np.arange(4))
    psel_np = np.equal.outer(np.arange(128) // 4, np.arange(32))

    nc = _build_program(bc, split_n, split_x, n_cores)

    bf = ml_dtypes.bfloat16
    cvec = np.concatenate([np.ones(split_n), -np.ones(HW6 - split_n)]).astype(np.float32)
    cful_np = np.repeat(cvec[None, :], 128, axis=0)
    cvex = np.concatenate([np.ones(split_x), -np.ones(HW6 - split_x)]).astype(np.float32)
    cfux_np = np.repeat(cvex[None, :], 128, axis=0)
    shared = {"w6n": w6n_np.astype(bf), "w6x": w6x_np.astype(bf),
              "wfcx": W_fcx.astype(bf), "wfcn": W_fcn.astype(bf),
              "mask": mask_np.astype(bf), "mask4": mask4_np.astype(bf),
              "psel": psel_np.astype(bf), "cful": cful_np, "cfux": cfux_np}
    in_maps = []
    for c in range(n_cores):
        ne_c = neibs[c * bc * NB:(c + 1) * bc * NB].astype(bf)
        in_maps.append({
            "ne": _permute_pmajor(ne_c, bc),
            "netm": ne_c,
            "xt": np.ascontiguousarray(x[c * bc:(c + 1) * bc].T).astype(bf),
            **shared,
        })
    global LAST_RESULTS
    res = run_bass_kernel_spmd(nc, in_maps, core_ids=list(range(n_cores)),
                               trace=TRACE, tmpdir=TRACE_DIR)
    LAST_RESULTS = res
    return np.concatenate([res.results[c]["out"] for c in range(n_cores)], axis=0)


# revision 28
# speedup vs baseline: 1.1418x; 1.1418x over previous
"""AttentionAggregator Trainium2 kernel (8-core SPMD, data-parallel over nodes).

Math (per node b with neighbors n):
  x_att   = lrelu_.01(x @ W_att);  neib_att = lrelu_.01(neibs @ W_att)
  e[b,n]  = lrelu_.2(x_att[b]@a_x + neib_att[b,n]@a_n)
  att     = softmax_n(e)
  agg[b]  = sum_n att[b,n] * neibs[b,n]
  out     = relu([x@W_fcx, agg@W_fcn])

Score rewrite (host-side, exact in the weights):
  sum_h a_h*lrelu(z_h) = sum_{seg1} relu(x.col) - sum_{seg2} relu(x.col)
  over 258 columns (relu-pair form), via lrelu(u)=.01u+.99relu(u),
  a*lrelu(z)=sign(a)*lrelu(|a|z), k*relu(u)=relu(k*u), u=relu(u)-relu(-u).

v2 design:
  - neibs cast to bf16 on host and laid out p-major per 128-node block so
    the natural load is fully contiguous per partition.
  - transposed neibs tiles come from the DMA xbar transpose (bf16) straight
    from HBM -> no PE transposes, no PSUM->SBUF tile copies.
  - x shipped pre-transposed bf16 from host (used for logits and fc).
  - all matmuls bf16 (FWL weight loads, 1 cycle/row): scores, agg, fc.
  - score relu+/-accumulate drains rotated across DVE/Pool/ACT (greedy
    load balance); softmax in a transposed [T,128] layout, bf16 where it
    doesn't hurt; outputs fp32.
"""
import warnings
warnings.filterwarnings("ignore")
import numpy as np
import ml_dtypes
from contextlib import ExitStack

import concourse.bass as bass
import concourse.tile as tile
from concourse import bacc, mybir, masks
from concourse.bass_utils import run_bass_kernel_spmd

F32 = mybir.dt.float32
BF16 = mybir.dt.bfloat16
AF = mybir.ActivationFunctionType
ALU = mybir.AluOpType
AX = mybir.AxisListType

N_CORES = 8
B_FULL, NB, D, H, O = 20000, 32, 128, 256, 128
HW6 = H + 2  # 258 score columns


def _score_weights(W_att: np.ndarray, a_half: np.ndarray):
    """Build the 258-column relu-pair score weight matrix. Returns (W6, split)."""
    pos = np.where(a_half >= 0)[0]
    neg = np.where(a_half < 0)[0]
    Wabs = W_att * np.abs(a_half)[None, :]
    w_d = (W_att @ a_half).astype(np.float64)
    seg1 = np.concatenate([0.99 * Wabs[:, pos], 0.01 * w_d[:, None]], axis=1)
    seg2 = np.concatenate([0.99 * Wabs[:, neg], -0.01 * w_d[:, None]], axis=1)
    W6 = np.concatenate([seg1, seg2], axis=1).astype(np.float32)
    return W6, seg1.shape[1]


def _blocks(bc):
    out = []
    o = 0
    while o < bc:
        f = min(128, bc - o)
        assert f * NB % 128 == 0
        out.append((o, f))
        o += f
    return out


_PROG_CACHE = {}

# test-harness knobs (harness calls kernel() with defaults: no tracing)
TRACE = False
TRACE_DIR = None
LAST_RESULTS = None


def _drain_engines(T):
    """Greedy per-tile drain-engine assignment balancing per-block load.

    "V": single fused drain on DVE (PSUM-capable).
    "A": two-segment Relu drain on ACT (accum+ / accum-).
    Pool has no PSUM port and no free-axis accumulate, so it only gets
    SBUF-side softmax work. Costs (ns) per drain; handicaps model each
    engine's other per-block duties.
    """
    load = {"V": 2000.0, "A": 1300.0}
    cost = {"V": 480.0, "A": 1175.0}
    plan = []
    for _ in range(T):
        e = min(load, key=lambda k: load[k] + cost[k])
        load[e] += cost[e]
        plan.append(e)
    return plan


def _build_program(bc, split_n, split_x, n_cores=N_CORES):
    """Build + compile the SPMD program for bc nodes per core."""
    key = (bc, split_n, split_x, n_cores)
    if key in _PROG_CACHE:
        return _PROG_CACHE[key]

    nc = bacc.Bacc("TRN2", target_bir_lowering=False, debug=False,
                   num_devices=n_cores)

    ne_d = nc.dram_tensor("ne", [bc * NB, D], BF16, kind="ExternalInput").ap()
    netm_d = nc.dram_tensor("netm", [bc * NB, D], BF16, kind="ExternalInput").ap()
    xt_d = nc.dram_tensor("xt", [D, bc], BF16, kind="ExternalInput").ap()
    w6n_d = nc.dram_tensor("w6n", [D, HW6], BF16, kind="ExternalInput").ap()
    w6x_d = nc.dram_tensor("w6x", [D, HW6], BF16, kind="ExternalInput").ap()
    wfcx_d = nc.dram_tensor("wfcx", [D, O], BF16, kind="ExternalInput").ap()
    wfcn_d = nc.dram_tensor("wfcn", [D, O], BF16, kind="ExternalInput").ap()
    mask_d = nc.dram_tensor("mask", [128, 4], BF16, kind="ExternalInput").ap()
    mask4_d = nc.dram_tensor("mask4", [128, 4], BF16, kind="ExternalInput").ap()
    psel_d = nc.dram_tensor("psel", [128, 32], BF16, kind="ExternalInput").ap()
    cful_d = nc.dram_tensor("cful", [128, HW6], F32, kind="ExternalInput").ap()
    cfux_d = nc.dram_tensor("cfux", [128, HW6], F32, kind="ExternalInput").ap()
    out_d = nc.dram_tensor("out", [bc, 2 * O], F32, kind="ExternalOutput").ap()

    with tile.TileContext(nc) as tc, ExitStack() as ctx:
        consts = ctx.enter_context(tc.tile_pool(name="consts", bufs=1))
        nepool = ctx.enter_context(tc.tile_pool(name="ne", bufs=3))
        ntpool = ctx.enter_context(tc.tile_pool(name="nt", bufs=3))
        xtpool = ctx.enter_context(tc.tile_pool(name="xtp", bufs=2))
        sc_v = ctx.enter_context(tc.tile_pool(name="scr_v", bufs=2))
        sc_p = ctx.enter_context(tc.tile_pool(name="scr_p", bufs=2))
        sc_a = ctx.enter_context(tc.tile_pool(name="scr_a", bufs=2))
        blkpool = ctx.enter_context(tc.tile_pool(name="blk", bufs=2))
        ps_sc = ctx.enter_context(tc.tile_pool(name="ps_sc", bufs=5, space="PSUM"))
        ps_agg = ctx.enter_context(tc.tile_pool(name="ps_agg", bufs=1, space="PSUM"))
        ps_misc = ctx.enter_context(tc.tile_pool(name="ps_misc", bufs=2, space="PSUM"))

        identf = consts.tile([128, 128], F32)
        masks.make_identity(nc, identf[:])
        ident = consts.tile([128, 128], BF16)
        nc.vector.tensor_copy(ident[:], identf[:])
        w6n = consts.tile([D, HW6], BF16)
        w6x = consts.tile([D, HW6], BF16)
        wfcx = consts.tile([D, O], BF16)
        wfcn = consts.tile([D, O], BF16)
        mask = consts.tile([128, 4], BF16)
        mask4 = consts.tile([128, 4], BF16)
        psel = consts.tile([128, 32], BF16)
        cful = consts.tile([128, HW6], F32)
        cfux = consts.tile([128, HW6], F32)
        for t, dd in [(w6n, w6n_d), (w6x, w6x_d), (wfcx, wfcx_d),
                      (wfcn, wfcn_d), (mask, mask_d), (mask4, mask4_d),
                      (psel, psel_d), (cful, cful_d), (cfux, cfux_d)]:
            nc.sync.dma_start(t[:], dd)

        def block_setup(boff, F):
            """DMAs + x-side for a block; returns block state."""
            T = F * NB // 128  # score tiles in this block
            rbase = boff * NB

            # natural (p-major host layout): partition p <- rows p*T..p*T+T
            ne_buf = nepool.tile([128, 32 * D], BF16, tag="ne")
            ne_v = ne_buf[:].rearrange("p (t d) -> p t d", d=D)
            nc.sync.dma_start(
                ne_v[:, :T, :],
                ne_d[rbase: rbase + 128 * T, :].rearrange(
                    "(p t) d -> p t d", t=T))

            # transposed via DMA xbar from the tile-major copy: SBUF col
            # (t*128+p) <- dram row 128t+p; out last dim (p) contiguous.
            nt_buf = ntpool.tile([128, 32 * 128], BF16, tag="nt")
            nt_v = nt_buf[:].rearrange("d (t p) -> d t p", p=128)
            nc.sync.dma_start_transpose(
                nt_v[:, :T, :], netm_d[rbase: rbase + 128 * T, :])

            # ---- x side (xT shipped pre-transposed bf16)
            xtr = xtpool.tile([D, 128], BF16, tag="xtr")
            nc.sync.dma_start(xtr[:, :F], xt_d[:, boff:boff + F])
            xs_ps = ps_misc.tile([128, 258], F32, tag="misc")
            nc.tensor.matmul(xs_ps[:F, :], xtr[:, :F], w6x[:], start=True, stop=True)
            xscr = sc_v.tile([128, HW6], F32, tag="scr_v")
            sx = blkpool.tile([128, 1], F32, tag="sx")
            nc.vector.scalar_tensor_tensor(
                xscr[:F, :], xs_ps[:F, :], 0.0, cfux[:F, :],
                op0=ALU.max, op1=ALU.mult, accum_out=sx[:F, :])
            sx4 = blkpool.tile([128, 4], BF16, tag="sx4")
            nc.gpsimd.tensor_scalar(sx4[:F, :], mask4[:F, :], sx[:F, 0:1], None,
                                    op0=ALU.mult)

            spos = blkpool.tile([128, 32], F32, tag="spos")
            sneg = blkpool.tile([128, 32], F32, tag="sneg")
            nc.gpsimd.memset(sneg[:, :T], 0.0)
            agg_ps = ps_agg.tile([128, 128], F32, tag="agg")
            return dict(ne_v=ne_v, nt_v=nt_v, xtr=xtr, T=T, F=F, boff=boff,
                        spos=spos, sneg=sneg, sx4=sx4, agg_ps=agg_ps)

        def emit_score(bs, t):
            s_ps = ps_sc.tile([128, HW6], F32, tag="sc")
            nc.tensor.matmul(s_ps[:], bs["nt_v"][:, t, :], w6n[:],
                             start=True, stop=True)
            if bs["plan"][t] == "V":
                scr = sc_v.tile([128, HW6], F32, tag="scr_v")
                nc.vector.scalar_tensor_tensor(
                    scr[:], s_ps[:], 0.0, cful[:],
                    op0=ALU.max, op1=ALU.mult,
                    accum_out=bs["spos"][:, t:t + 1])
            else:
                scr = sc_a.tile([128, HW6], BF16, tag="scr_a")
                nc.scalar.activation(scr[:, :split_n], s_ps[:, :split_n],
                                     AF.Relu, accum_out=bs["spos"][:, t:t + 1])
                nc.scalar.activation(scr[:, split_n:HW6],
                                     s_ps[:, split_n:HW6], AF.Relu,
                                     accum_out=bs["sneg"][:, t:t + 1])

        def emit_agg(bs, t, a_all_h, t0):
            a_v = a_all_h[:].rearrange("p (t j) -> p t j", j=4)
            nc.tensor.matmul(bs["agg_ps"][:, 4 * t:4 * (t + 1)],
                             bs["ne_v"][:, t, :], a_v[:, t - t0, :],
                             start=True, stop=True)

        def softmax_half(bs, t0, TH):
            """Softmax for tiles [t0, t0+TH); returns a_all for the half."""
            F, spos, sneg = bs["F"], bs["spos"], bs["sneg"]
            sxg_ps = ps_misc.tile([128, 258], F32, tag="misc")
            nc.tensor.matmul(sxg_ps[:TH, 0:4], psel[:F, t0:t0 + TH],
                             bs["sx4"][:F, :], start=True, stop=True)
            sxg = blkpool.tile([16, 4], F32, tag="sxg")
            nc.vector.tensor_copy(sxg[:TH, :], sxg_ps[:TH, 0:4])
            s_col = blkpool.tile([128, 16], BF16, tag="s_col")
            nc.gpsimd.tensor_tensor(s_col[:, :TH], spos[:, t0:t0 + TH],
                                    sneg[:, t0:t0 + TH], op=ALU.subtract)
            snt_ps = ps_misc.tile([128, 258], BF16, tag="misc")
            nc.tensor.transpose(snt_ps[:TH, :128], s_col[:, :TH], ident[:])
            z = blkpool.tile([16, 128], F32, tag="z")
            nc.vector.tensor_tensor(
                z[:TH, :].rearrange("t (j n) -> t j n", n=32),
                snt_ps[:TH, :128].rearrange("t (j n) -> t j n", n=32),
                sxg[:TH, :].unsqueeze(2).broadcast_to([TH, 4, 32]),
                op=ALU.add)
            zl = blkpool.tile([16, 128], F32, tag="zl")
            nc.vector.scalar_tensor_tensor(zl[:TH, :], z[:TH, :], 0.2,
                                           z[:TH, :], op0=ALU.mult, op1=ALU.max)
            ex = blkpool.tile([16, 128], F32, tag="ex")
            nc.scalar.activation(ex[:TH, :], zl[:TH, :], AF.Exp)
            sums = blkpool.tile([16, 4], F32, tag="sums")
            nc.vector.tensor_reduce(
                sums[:TH, :], ex[:TH, :].rearrange("t (j n) -> t j n", n=32),
                axis=AX.X, op=ALU.add)
            rec = blkpool.tile([16, 4], F32, tag="rec")
            nc.vector.reciprocal(rec[:TH, :], sums[:TH, :])
            att = blkpool.tile([16, 128], BF16, tag="att")
            nc.gpsimd.tensor_tensor(
                att[:TH, :].rearrange("t (j n) -> t j n", n=32),
                ex[:TH, :].rearrange("t (j n) -> t j n", n=32),
                rec[:TH, :].unsqueeze(2).broadcast_to([TH, 4, 32]),
                op=ALU.mult)
            att_ps = ps_misc.tile([128, 258], BF16, tag="misc")
            nc.tensor.transpose(att_ps[:, :TH], att[:TH, :], ident[:TH, :TH])
            a_all = blkpool.tile([128, 64], BF16, tag="a_all")
            nc.vector.tensor_tensor(
                a_all[:].rearrange("p (t j) -> p t j", j=4)[:, :TH, :],
                mask[:].unsqueeze(1).broadcast_to([128, TH, 4]),
                att_ps[:, :TH].unsqueeze(2).broadcast_to([128, TH, 4]),
                op=ALU.mult)
            return a_all

        def block_finish(bs):
            F, boff = bs["F"], bs["boff"]
            aggt = blkpool.tile([D, 128], BF16, tag="aggt")
            nc.scalar.copy(aggt[:, :F], bs["agg_ps"][:, :F])
            fc_ps = ps_misc.tile([128, 258], F32, tag="misc")
            nc.tensor.matmul(fc_ps[:F, 0:O], bs["xtr"][:, :F], wfcx[:],
                             start=True, stop=True)
            nc.tensor.matmul(fc_ps[:F, O:2 * O], aggt[:, :F], wfcn[:],
                             start=True, stop=True)
            out_sb = blkpool.tile([128, 2 * O], F32, tag="out")
            nc.scalar.activation(out_sb[:F, :], fc_ps[:F, :2 * O], AF.Relu)
            nc.sync.dma_start(out_d[boff:boff + F, :], out_sb[:F, :])

        # ---- half-block software pipeline
        def halves(T):
            h0 = (T + 1) // 2
            return [(0, h0), (h0, T - h0)]

        blocks = _blocks(bc)
        steps = []  # (block_state, t0, TH)
        for (boff, F) in blocks:
            T = F * NB // 128
            bs = None  # created lazily at first half
            for (t0, TH) in halves(T):
                steps.append((boff, F, t0, TH))

        bs_by_off = {}
        pend_smax = None   # (bs, t0, TH) awaiting softmax emission
        pend_agg = None    # (bs, t0, TH, a_all) awaiting agg interleave
        for (boff, F, t0, TH) in steps:
            if boff not in bs_by_off:
                bs = block_setup(boff, F)
                bs["plan"] = _drain_engines(bs["T"])
                bs["done_halves"] = 0
                bs_by_off = {boff: bs}  # keep only current (states referenced in pend_*)
            bs = bs_by_off[boff]
            agg_job = pend_agg
            pend_agg = None
            for i in range(TH):
                emit_score(bs, t0 + i)
                if agg_job is not None:
                    abs_, at0, aTH, a_all = agg_job
                    if i < aTH:
                        emit_agg(abs_, at0 + i, a_all, at0)
                    if i == TH - 1:
                        for j in range(TH, aTH):
                            emit_agg(abs_, at0 + j, a_all, at0)
            if agg_job is not None:
                abs_, at0, aTH, _ = agg_job
                abs_["done_halves"] += 1
                if abs_["done_halves"] == 2:
                    block_finish(abs_)
            if pend_smax is not None:
                sbs, st0, sTH = pend_smax
                a_all = softmax_half(sbs, st0, sTH)
                pend_agg = (sbs, st0, sTH, a_all)
            pend_smax = (bs, t0, TH)
        # drain the pipeline tail
        if pend_smax is not None:
            sbs, st0, sTH = pend_smax
            a_all = softmax_half(sbs, st0, sTH)
            if pend_agg is not None:
                abs_, at0, aTH, pa = pend_agg
                for j in range(aTH):
                    emit_agg(abs_, at0 + j, pa, at0)
                abs_["done_halves"] += 1
                if abs_["done_halves"] == 2:
                    block_finish(abs_)
            for j in range(sTH):
                emit_agg(sbs, st0 + j, a_all, st0)
            sbs["done_halves"] += 1
            if sbs["done_halves"] == 2:
                block_finish(sbs)

    nc.compile()
    _PROG_CACHE[key] = nc
    return nc


def _permute_pmajor(ne_c: np.ndarray, bc: int) -> np.ndarray:
    """Per 128-node block, reorder rows tile-major -> partition-major."""
    chunks = []
    r = 0
    for (boff, F) in _blocks(bc):
        T = F * NB // 128
        blk = ne_c[r:r + 128 * T]  # rows ordered (t, p)
        chunks.append(blk.reshape(T, 128, D).transpose(1, 0, 2).reshape(-1, D))
        r += 128 * T
    return np.concatenate(chunks, axis=0)


def kernel(x, neibs, W_att, W_fcx, W_fcn, a, n_cores=N_CORES):
    x = np.asarray(x, dtype=np.float32)
    neibs = np.asarray(neibs, dtype=np.float32)
    W_att = np.asarray(W_att, dtype=np.float32)
    W_fcx = np.asarray(W_fcx, dtype=np.float32)
    W_fcn = np.asarray(W_fcn, dtype=np.float32)
    a = np.asarray(a, dtype=np.float32)

    B = x.shape[0]
    bc = B // n_cores
    a_x, a_n = a[:H, 0], a[H:, 0]
    w6x_np, split_x = _score_weights(W_att, a_x)
    w6n_np, split_n = _score_weights(W_att, a_n)
    mask_np = np.equal.outer(np.arange(128) // 32, np.arange(4))
    mask4_np = np.equal.outer(np.arange(128) % 4, np.arange(4))
    psel_np = np.equal.outer(np.arange(128) // 4, np.arange(32))

    nc = _build_program(bc, split_n, split_x, n_cores)

    bf = ml_dtypes.bfloat16
    cvec = np.concatenate([np.ones(split_n), -np.ones(HW6 - split_n)]).astype(np.float32)
    cful_np = np.repeat(cvec[None, :], 128, axis=0)
    cvex = np.concatenate([np.ones(split_x), -np.ones(HW6 - split_x)]).astype(np.float32)
    cfux_np = np.repeat(cvex[None, :], 128, axis=0)
    shared = {"w6n": w6n_np.astype(bf), "w6x": w6x_np.astype(bf),
              "wfcx": W_fcx.astype(bf), "wfcn": W_fcn.astype(bf),
              "mask": mask_np.astype(bf), "mask4": mask4_np.astype(bf),
              "psel": psel_np.astype(bf), "cful": cful_np, "cfux": cfux_np}
    in_maps = []
    for c in range(n_cores):
        ne_c = neibs[c * bc * NB:(c + 1) * bc * NB].astype(bf)
        in_maps.append({
            "ne": _permute_pmajor(ne_c, bc),
            "netm": ne_c,
            "xt": np.ascontiguousarray(x[c * bc:(c + 1) * bc].T).astype(bf),
            **shared,
        })
    global LAST_RESULTS
    res = run_bass_kernel_spmd(nc, in_maps, core_ids=list(range(n_cores)),
                               trace=TRACE, tmpdir=TRACE_DIR)
    LAST_RESULTS = res
    return np.concatenate([res.results[c]["out"] for c in range(n_cores)], axis=0)
